# revision 93
# baseline (speedup 1.0000x reference)
"""Trainium2 Bass kernel for nn_FCGAT (fully-connected GAT variant).

Mathematical simplifications (exact, same as the v1 kernel):

1. ``einsum('nkj,nkd->nkd', softmax(aa,2), h) == h`` (softmax rows sum to 1),
   so the whole attention block is dead code and the model reduces to::

       h_s = lrelu(lrelu([towers | x_s] @ w1.T + b1) @ w2.T + b2)
       x_{s+1} = h_s + x_s
       out_n = prod_k sigmoid(x_K @ ow[0] + ob[0])

2. Residuals are distributed through the linear maps as PSUM accumulation
   groups: ``x_s = x_0 + sum_{t<s} h_t`` so no elementwise adds exist.

v2 speedups over the 15.6us v1 kernel (CoreSim cost model):

* bf16 everywhere on the data path (inputs rounded on host, weights
  pre-rounded, hidden activations stored bf16; PSUM stays f32).  Halves the
  input DMA and makes every matmul 1 PE cycle/row.  Measured end-to-end
  numeric error vs a float64 oracle: 5.5e-3 (budget 2e-2).
* host-side transpose of the input to feature-major [D1, rows] removes the
  8 on-device PE transposes + DVE rounding copies of v1 entirely.
* 3 input DMAs ride 3 different queues (SP / ACT HWDGE + Pool SWDGE) which
  the cost model executes fully in parallel: all input sems fire at ~2.4us
  (the fixed DMA latency floor).
* Prelu (== leaky_relu with alpha) replaces Lrelu: parametric_relu lives in
  the SAME activation-table set as Sigmoid, so ONE table load (prewarmed at
  t=0 under the DMA shadow) covers the whole kernel.  v1 paid 2 prewarm
  loads plus a 1.3us mid-kernel reload for the final sigmoid.
* the logits matmul is restructured: instead of [1, R] output rows on one
  partition (v1: 612ns+ sigmoid, then a 1.5us DVE product tree), each
  64-row graph becomes a *stationary* operand against the moving ow column,
  yielding logits [64, 16] with one graph per psum column.  Sigmoid is then
  a 198ns ACT op; one [64,16] PE transpose + one DVE cumprod scan
  (tensor_tensor_scan, op0=mult) put the 16 per-graph products on 16
  partitions, written out by a 16-descriptor DMA.
* the ACT engine saturates the middle of the kernel (8 prelu tiles), so the
  last DVE_COLS/DVE_H1_COLS columns of every h1/h2 chunk are offloaded to
  the otherwise-idle DVE as max(x+b, 0.01*(x+b)) (tensor_scalar +
  scalar_tensor_tensor); the last step's h2 offload shrinks per chunk
  (dve_last=(128, 64)) so the ACT chain and the DVE pair chain converge at
  the same moment instead of the tail waiting on the slower DVE path.  CRITICAL HW CONSTRAINT discovered via isolated
  probes: concurrent ACT+DVE reads of the SAME PSUM bank hang the device
  (NRT timeout), while the CoreSim cost model is happy to overlap them.
  The mm1/mm2 matmuls are therefore split so the ACT-read and DVE-read
  slices land in different PSUM banks (the DVE h2 slices borrow the
  late-kernel logits/transpose banks; h1 gets a dedicated bank pair),
  using all 8 banks.

Sharding: data-parallel over N=128 -> 16 graphs (1024 rows) per core across
8 NeuronCores; weights replicated.

Raw Bass (explicit engine blocks + standalone single-condition waits), same
rationale as v1: the walrus build allows only one sync-wait per instruction.
"""

from contextlib import ExitStack

import ml_dtypes
import numpy as np

import concourse.bass as bass
import concourse.mybir as mybir
from concourse.bass_utils import run_bass_kernel_spmd

N_CORES = 8
N, K, DT, D2 = 128, 64, 64, 64
D1 = DT + D2                # 128: [towers | x] feature dim
G = N // N_CORES            # 16 graphs per core
R = G * K                   # 1024 rows per core
CHUNK = 512                 # psum-bank / moving-operand chunk
NCHUNK = R // CHUNK         # 2
TILE = 128                  # logits stationary tile (rows)
NT = R // TILE              # 8 logits tiles -> psum [128, NT]

# packed constants layout (f32 columns of a [128, CW] array; bf16 payloads
# are stored as raw bytes and bitcast on device)
C_ID = 0                    # 0:128    identity (f32, final transpose)
C_W1B = 128                 # 128:192  w1.T as bf16 [128, 128]
C_W2B = 192                 # 192:224  w2.T as bf16 [128, 64]
C_OWB = 224                 # col 224, rows 64:128: ow[0] as bf16 (low half)
C_B1 = 225                  # b1 f32 column
C_B2 = 226                  # b2 f32 column, rows 64:128
C_OB = 227                  # ob f32 replicated on all 128 rows
C_Z = 228                   # 228:292  zeros (f32; scan dummy + output zeroing)
CW = 292

_F32 = mybir.dt.float32
_BF16 = mybir.dt.bfloat16

# FAST_OUT (SWDGE scatter-add + trigger output path) cuts ~2.1us off the
# CoreSim tail but the gpsimd dma_scatter_add Q7 kernel crashes the device
# on this terminal's firmware (verified with isolated probes in both
# prepare/trigger and immediate modes; a plain load_library works).  Keep
# the implementation but ship with the plain HWDGE output DMA.
FAST_OUT = False
DVE_COLS = 128
DVE_H1_COLS = 128

LAST_RESULT = None
_PROGRAM_CACHE = {}


def _build_program(kk: int, act_fn=None, sig_fn=None, out_sync=2,
                   fast_out=FAST_OUT, dve_cols=DVE_COLS,
                   dve_h1_cols=None, dve_last=(128, 64)) -> bass.Bass:
    """act_fn/sig_fn overrides exist for CoreSim exec-mode validation
    (CoreSim implements Relu/Sigmoid but not Prelu).

    out_sync: 2 = output DMA incs sem_out and SP waits on it (fully safe);
    1 = inc only, no wait; 0 = no completion sem at all.

    dve_cols: offload the last dve_cols columns of every h2 chunk from the
    saturated ACT engine to the otherwise-idle DVE as
    max(x + b2, 0.01*(x + b2)) (two DVE ops).  Must be a multiple of 64 so
    the per-graph logits slices don't straddle the ACT/DVE boundary.

    fast_out: write the 16 products with a SWDGE scatter-add whose
    descriptors are prepared at t~1us and fired by trigger_dma at the end.
    The trigger path skips the 625ns HWDGE + 650ns DGE delay of a regular
    DMA, cutting ~1.2us off the tail.  Scatter-add ACCUMULATES into HBM, so
    the output buffer is zeroed by an early DMA (completion-sem ordered
    before the trigger).  fast_out=False falls back to a plain SP-queue
    DMA."""
    PRELU = act_fn or mybir.ActivationFunctionType.Prelu
    SIGMOID = sig_fn or mybir.ActivationFunctionType.Sigmoid
    if dve_h1_cols is None:
        dve_h1_cols = DVE_H1_COLS
    DD = dve_cols                   # DVE columns per h2 chunk
    HH = dve_h1_cols                # DVE columns per h1 chunk
    AH = CHUNK - HH                 # ACT columns per h1 chunk
    assert DD % 64 == 0 and 0 <= DD < CHUNK
    assert HH % 64 == 0 and 0 <= HH < CHUNK
    _dl = dve_last if isinstance(dve_last, tuple) else (dve_last,)
    assert all(v % 64 == 0 and 0 <= v <= DD for v in _dl)

    def ddf(s, c=0):
        # the last step's DVE pairs gate the tail; shrink them (per chunk)
        # so the ACT chain and the DVE pair chain converge together
        if kk == 0 or s < kk - 1:
            return DD
        return dve_last[c] if isinstance(dve_last, tuple) else dve_last

    def aaf(s, c=0):
        return CHUNK - ddf(s, c)

    nc = bass.Bass()
    const_d = nc.declare_dram_parameter("cpack", [128, CW], _F32,
                                        isOutput=False)
    xd = nc.declare_dram_parameter("xc0T", [128, R], _BF16, isOutput=False)
    out_d = nc.declare_dram_parameter("out", [G, 64 if fast_out else 1],
                                      _F32, isOutput=True)
    idx_d = (nc.declare_dram_parameter("idxs", [128, 8], mybir.dt.int16,
                                       isOutput=False) if fast_out else None)

    with ExitStack() as ctx:
        cs = ctx.enter_context(nc.sbuf_tensor([128, CW], _F32))
        idxs_sb = ctx.enter_context(
            nc.sbuf_tensor("idxs_sb", [128, 8], mybir.dt.int16))
        tmp = ctx.enter_context(
            nc.sbuf_tensor("tmp", [128, max(1, 2 * kk * max(DD, 1))], _BF16))
        tmph = ctx.enter_context(
            nc.sbuf_tensor("tmph", [128, max(1, 2 * kk * max(HH, 1))],
                           _BF16))
        xcT = ctx.enter_context(nc.sbuf_tensor([128, R], _BF16))
        h1s = ctx.enter_context(nc.sbuf_tensor([128, R], _BF16))
        h2s = [ctx.enter_context(nc.sbuf_tensor(f"h2s_{s}", [128, R], _BF16))
               for s in range(kk)]
        sig = ctx.enter_context(nc.sbuf_tensor([128, G], _F32))
        prod = ctx.enter_context(nc.sbuf_tensor([128, K], _F32))
        warm = ctx.enter_context(nc.sbuf_tensor([1, 1], _F32))
        # full-bank psum allocations (avoid same-bank PE-write/engine-read)
        ps1 = [ctx.enter_context(nc.psum_tensor(f"ps1_{c}", [128, 512], _F32))
               for c in range(NCHUNK)]
        ps2 = [ctx.enter_context(nc.psum_tensor(f"ps2_{c}", [128, 512], _F32))
               for c in range(NCHUNK)]
        ps3 = ctx.enter_context(nc.psum_tensor([128, 512], _F32))
        pst = ctx.enter_context(nc.psum_tensor([128, 512], _F32))
        psb = [ps3, pst]        # DVE-read h2 slices borrow these banks
        ps1b = [ctx.enter_context(
            nc.psum_tensor(f"ps1b_{c}", [128, 512], _F32))
            for c in range(NCHUNK if HH else 0)]
        sem_const = ctx.enter_context(nc.semaphore("sem_const"))
        sem_d = [ctx.enter_context(nc.semaphore(f"sem_d{c}"))
                 for c in range(NCHUNK)]
        sem_out = ctx.enter_context(nc.semaphore("sem_out"))
        sem_zero = ctx.enter_context(nc.semaphore("sem_zero"))
        sem_prep = ctx.enter_context(nc.semaphore("sem_prep"))
        sem_idx = ctx.enter_context(nc.semaphore("sem_idx"))
        pe_sem = ctx.enter_context(nc.semaphore("pe_sem"))
        act_sem = ctx.enter_context(nc.semaphore("act_sem"))
        dve_sem = ctx.enter_context(nc.semaphore("dve_sem"))
        block = ctx.enter_context(nc.Block())

        ident = cs[:, C_ID:C_ID + 128]
        w1t = cs[:, C_W1B:C_W1B + 64].bitcast(_BF16)        # [128, 128]
        w2t = cs[:, C_W2B:C_W2B + 32].bitcast(_BF16)        # [128, 64]
        owc = cs[DT:D1, C_OWB:C_OWB + 1].bitcast(_BF16)[:, 0:1]  # [64, 1]
        b1 = cs[:, C_B1:C_B1 + 1]
        b2 = cs[DT:D1, C_B2:C_B2 + 1]
        obc = cs[:, C_OB:C_OB + 1]
        zrow = cs[0:G, C_Z:C_Z + K]                         # [16, 64] zeros

        def csl(c):
            return slice(c * CHUNK, (c + 1) * CHUNK)

        # ---- PE instruction numbering (emitted below in this order) ----
        pe_n = 0
        pe_mm1 = {}   # (s, c) -> pe value after the mm1 group for (s, c)
        pe_mm1b = {}  # (s, c) DVE-bank h1 slice
        pe_mm2 = {}   # (s, c)
        pe_mm2b = {}  # (s, c) DVE-bank slice
        pe_mm3_last = 0
        pe_transpose = 0
        act_n = 1     # warm == 1
        act_h1 = {}
        act_h2 = {}
        for s in range(kk):
            for c in range(NCHUNK):
                act_h1[(s, c)] = act_n + 1
                act_n += 1
            for c in range(NCHUNK):
                act_h2[(s, c)] = act_n + 1
                act_n += 1
        act_sig = act_n + 1
        # DVE numbering: memset (fast_out), then per step the h1-offload
        # pairs then the h2-offload pairs, then the cumprod scan
        dve_n = 1 if fast_out else 0
        dve_h1b = {}
        dve_h2b = {}
        for s in range(kk):
            for c in range(NCHUNK):
                if HH:
                    dve_n += 2
                    dve_h1b[(s, c)] = dve_n
            for c in range(NCHUNK):
                if ddf(s, c):
                    dve_n += 2
                    dve_h2b[(s, c)] = dve_n
        dve_scan = dve_n + 1

        @block.sync
        def _(sync):
            sync.dma_start(xcT[:, csl(0)], xd[:, csl(0)]).then_inc(
                sem_d[0], 16)
            if fast_out:
                sync.dma_start(idxs_sb[:, :], idx_d[:, :]).then_inc(
                    sem_idx, 16)
            if not fast_out:
                sync.wait_ge(dve_sem, dve_scan)
                dma = sync.dma_start(out_d[:, :], prod[0:G, K - 1:K])
                if out_sync >= 1:
                    dma.then_inc(sem_out, 16)
            if out_sync >= 2:
                sync.wait_ge(sem_out, 16)

        @block.tensor
        def _(tensor):
            nonlocal pe_mm3_last, pe_transpose
            wm = {}

            def twait(sem, val):
                if wm.get(id(sem), 0) < val:
                    wm[id(sem)] = val
                    tensor.wait_ge(sem, val)

            pe = 0

            def inc(instr):
                nonlocal pe
                pe += 1
                instr.then_inc(pe_sem, 1)

            def mm3_column(g):
                # one complete accumulation group per psum column (one GRAPH
                # per column: 64 rows -> 64 output partitions): the PSUM
                # start bit zeroes the whole 2KB bank region, so groups in a
                # bank must be strictly sequential, not interleaved
                gsl = slice(g * K, (g + 1) * K)
                terms = [xcT[DT:D1, gsl]] + [
                    h2s[s][DT:D1, gsl] for s in range(kk)]
                for i, lhsT in enumerate(terms):
                    inc(nc.tensor.matmul(
                        ps3[0:K, g:g + 1], lhsT, owc,
                        start=(i == 0), stop=(i == len(terms) - 1),
                    ))

            def mm1_group(s, c, is_first):
                # moving columns split: group-a (ACT's bank), group-b
                # (DVE's bank) -- same PSUM-contention fix as mm2
                lo = c * CHUNK
                parts = [(ps1[c][:, 0:AH], slice(lo, lo + AH))]
                if HH:
                    parts.append(
                        (ps1b[c][:, 0:HH], slice(lo + AH, lo + CHUNK)))
                pes = []
                for dst, msl in parts:
                    inc(nc.tensor.matmul(
                        dst, w1t, xcT[:, msl],
                        start=True, stop=is_first))
                    pes.append(pe)
                return pes

            def mm1_h2part(s, c, t, last):
                lo = c * CHUNK
                parts = [(ps1[c][:, 0:AH], slice(lo, lo + AH))]
                if HH:
                    parts.append(
                        (ps1b[c][:, 0:HH], slice(lo + AH, lo + CHUNK)))
                pes = []
                for dst, msl in parts:
                    inc(nc.tensor.matmul(
                        dst, w1t[DT:D1, :], h2s[t][DT:D1, msl],
                        start=False, stop=last))
                    pes.append(pe)
                return pes

            # step-0 mm1
            twait(sem_const, 16)
            for c in range(NCHUNK):
                twait(sem_d[c], 16)
                pes = mm1_group(0, c, is_first=True)
                pe_mm1[(0, c)] = pes[0]
                pe_mm1b[(0, c)] = pes[-1]
            for s in range(kk):
                last_step = (s == kk - 1)
                for c in range(NCHUNK):
                    lo = c * CHUNK
                    twait(act_sem, act_h1[(s, c)])
                    if HH:
                        twait(dve_sem, dve_h1b[(s, c)])
                    if s >= 1 and ddf(s - 1, c):
                        # DVE bank WAR vs previous step's DVE h2 reader
                        twait(dve_sem, dve_h2b[(s - 1, c)])
                    if ddf(s, c):
                        # concurrent ACT+DVE reads of one PSUM bank hang
                        # the device (verified by an isolated probe), so
                        # mm2 splits: the ACT slice lands in ps2[c], the
                        # DVE slice in the late-kernel ps3/pst banks
                        aa, dd = aaf(s, c), ddf(s, c)
                        inc(nc.tensor.matmul(
                            ps2[c][DT:D1, 0:aa], w2t, h1s[:, lo:lo + aa],
                            start=True, stop=True))
                        pe_mm2[(s, c)] = pe
                        inc(nc.tensor.matmul(
                            psb[c][DT:D1, CHUNK - dd:CHUNK], w2t,
                            h1s[:, lo + aa:lo + CHUNK],
                            start=True, stop=True))
                        pe_mm2b[(s, c)] = pe
                    else:
                        inc(nc.tensor.matmul(
                            ps2[c][DT:D1, :], w2t, h1s[:, csl(c)],
                            start=True, stop=True))
                        pe_mm2[(s, c)] = pe
                    if not last_step:
                        # next-step mm1 x0 part; ps1[c]/ps1b[c] WAR cleared
                        # by the act_h1/dve_h1b waits just above
                        mm1_group(s + 1, c, is_first=False)
                if not last_step:
                    for c in range(NCHUNK):
                        for t in range(s):
                            mm1_h2part(s + 1, c, t, last=False)
                        # term t == s arrives in two pieces when the h2
                        # activation is split across ACT and DVE; wait for
                        # both producers
                        twait(act_sem, act_h2[(s, c)])
                        if ddf(s, c):
                            twait(dve_sem, dve_h2b[(s, c)])
                        pes = mm1_h2part(s + 1, c, s, last=True)
                        pe_mm1[(s + 1, c)] = pes[0]
                        pe_mm1b[(s + 1, c)] = pes[-1]
            for c in range(NCHUNK):
                if kk > 0:
                    twait(act_sem, act_h2[(kk - 1, c)])
                    if ddf(kk - 1, c):
                        twait(dve_sem, dve_h2b[(kk - 1, c)])
                for g in range(c * (G // NCHUNK), (c + 1) * (G // NCHUNK)):
                    mm3_column(g)
            pe_mm3_last = pe
            twait(act_sem, act_sig)
            inc(nc.tensor.transpose(
                pst[0:G, 0:K], sig[0:K, 0:G], ident[0:K, 0:K]))
            pe_transpose = pe

        @block.scalar
        def _(scalar):
            scalar.dma_start(xcT[:, csl(1)], xd[:, csl(1)]).then_inc(
                sem_d[1], 16)
            zcell = nc.const_aps.aps[(mybir.dt.float32, 0.0)][0:1, 0:1]
            nc.scalar.activation(warm[0:1, 0:1], zcell, SIGMOID).then_inc(
                act_sem, 1)
            seen = 0

            def swait(val):
                nonlocal seen
                if val > seen:
                    seen = val
                    scalar.wait_ge(pe_sem, val)

            for s in range(kk):
                for c in range(NCHUNK):
                    lo = c * CHUNK
                    swait(pe_mm1[(s, c)])
                    nc.scalar.activation(
                        h1s[:, lo:lo + AH], ps1[c][:, 0:AH], PRELU,
                        bias=b1, alpha=0.01,
                    ).then_inc(act_sem, 1)
                for c in range(NCHUNK):
                    lo = c * CHUNK
                    swait(pe_mm2[(s, c)])
                    aa = aaf(s, c)
                    nc.scalar.activation(
                        h2s[s][DT:D1, lo:lo + aa], ps2[c][DT:D1, 0:aa],
                        PRELU, bias=b2, alpha=0.01,
                    ).then_inc(act_sem, 1)
            swait(pe_mm3_last)
            nc.scalar.activation(
                sig[0:K, 0:G], ps3[0:K, 0:G], SIGMOID, bias=obc[0:K, :],
            ).then_inc(act_sem, 1)

        @block.vector
        def _(vector):
            if fast_out:
                # the scatter-add src AP spans all 128 partitions; only
                # 0:G hold real products, the rest must be initialized
                # (they map to ignored negative indices).  Engines need
                # quadrant-aligned base partitions, so clear the full
                # column up front and let the scan overwrite rows 0:G.
                nc.vector.memset(prod[:, K - 1:K], 0.0).then_inc(
                    dve_sem, 1)
            # lrelu offload pairs: out = max(x + b, 0.01*(x + b))
            for s in range(kk):
                for c in range(NCHUNK):
                    if not HH:
                        continue
                    lo = c * CHUNK
                    tsl = slice((s * NCHUNK + c) * HH,
                                (s * NCHUNK + c + 1) * HH)
                    vector.wait_ge(pe_sem, pe_mm1b[(s, c)])
                    nc.vector.tensor_scalar(
                        out=tmph[:, tsl], in0=ps1b[c][:, 0:HH],
                        scalar1=b1, scalar2=0.01,
                        op0=mybir.AluOpType.add, op1=mybir.AluOpType.mult,
                    ).then_inc(dve_sem, 1)
                    vector.wait_ge(dve_sem, dve_h1b[(s, c)] - 1)
                    nc.vector.scalar_tensor_tensor(
                        out=h1s[:, lo + AH:lo + CHUNK],
                        in0=ps1b[c][:, 0:HH], scalar=b1,
                        in1=tmph[:, tsl],
                        op0=mybir.AluOpType.add, op1=mybir.AluOpType.max,
                    ).then_inc(dve_sem, 1)
                for c in range(NCHUNK):
                    dd = ddf(s, c)
                    if not dd:
                        continue
                    aa = aaf(s, c)
                    lo = c * CHUNK
                    tsl = slice((s * NCHUNK + c) * DD,
                                (s * NCHUNK + c) * DD + dd)
                    vector.wait_ge(pe_sem, pe_mm2b[(s, c)])
                    nc.vector.tensor_scalar(
                        out=tmp[DT:D1, tsl],
                        in0=psb[c][DT:D1, CHUNK - dd:CHUNK],
                        scalar1=b2, scalar2=0.01,
                        op0=mybir.AluOpType.add, op1=mybir.AluOpType.mult,
                    ).then_inc(dve_sem, 1)
                    # explicit completion wait: DVE RAW on tmp
                    vector.wait_ge(dve_sem, dve_h2b[(s, c)] - 1)
                    nc.vector.scalar_tensor_tensor(
                        out=h2s[s][DT:D1, lo + aa:lo + CHUNK],
                        in0=psb[c][DT:D1, CHUNK - dd:CHUNK], scalar=b2,
                        in1=tmp[DT:D1, tsl],
                        op0=mybir.AluOpType.add, op1=mybir.AluOpType.max,
                    ).then_inc(dve_sem, 1)
            # explicit ordering: the scan below rewrites rows 0:G of the
            # memset column
            vector.wait_ge(dve_sem, dve_scan - 1)
            vector.wait_ge(pe_sem, pe_transpose)
            nc.vector.tensor_tensor_scan(
                prod[0:G, 0:K], pst[0:G, 0:K], zrow,
                initial=1.0,
                op0=mybir.AluOpType.mult, op1=mybir.AluOpType.bypass,
            ).then_inc(dve_sem, 1)

        @block.gpsimd
        def _(gpsimd):
            gpsimd.dma_start(cs[:, :], const_d[:, :]).then_inc(sem_const, 16)
            if fast_out:
                from concourse import library_config
                nc.gpsimd.load_library(library_config.mlp)
                # zero the scatter-add target (completion-sem ordered
                # before the trigger below); reads the cpack zeros region
                gpsimd.wait_ge(sem_const, 16)
                gpsimd.dma_start(out_d[:, :], zrow).then_inc(sem_zero, 16)
                gpsimd.wait_ge(sem_idx, 16)
                nc.gpsimd.dma_scatter_add(
                    out_ap=out_d[:, 0:1],
                    in_ap=prod[:, K - 1:K],
                    idxs_ap=idxs_sb[:, :],
                    num_idxs=128,
                    num_idxs_reg=G,
                    elem_size=1,
                    elem_step=64,
                    prepare_only=True,
                    sem=sem_out,
                ).then_inc(sem_prep, 1)
                gpsimd.wait_ge(sem_prep, 1)
                gpsimd.wait_ge(sem_zero, 16)
                gpsimd.wait_ge(dve_sem, dve_scan)
                gpsimd.trigger_dma(1)

    return nc


def _pack_consts(w1, b1, w2, b2, ow, ob):
    cp = np.zeros((128, CW), np.float32)
    cp[:, C_ID:C_ID + 128] = np.eye(128, dtype=np.float32)
    bv = cp.view(np.uint8)
    bf = ml_dtypes.bfloat16
    bv[:, C_W1B * 4:C_W1B * 4 + 256] = np.ascontiguousarray(
        w1.T.astype(bf)).view(np.uint8)
    bv[:, C_W2B * 4:C_W2B * 4 + 128] = np.ascontiguousarray(
        w2.T.astype(bf)).view(np.uint8)
    bv[DT:D1, C_OWB * 4:C_OWB * 4 + 2] = np.ascontiguousarray(
        ow.reshape(D2, 1).astype(bf)).view(np.uint8)
    cp[:, C_B1] = b1
    cp[DT:D1, C_B2] = b2
    cp[:, C_OB] = np.float32(ob.reshape(())[()])
    return cp


def _scatter_idxs():
    # scatter-add index table: position j (column-major over [16, ncols])
    # maps input partition j; the first G positions target output row g=j,
    # the rest are the ignored -1
    idx = np.full((128, 8), -1, np.int16)
    idx[0:G, 0] = np.arange(G, dtype=np.int16)
    return idx


def _make_in_maps(towers, x, w1, b1, w2, b2, ow, ob):
    towers = np.asarray(towers, np.float32)
    x = np.asarray(x, np.float32)
    cpack = _pack_consts(
        np.asarray(w1, np.float32), np.asarray(b1, np.float32),
        np.asarray(w2, np.float32), np.asarray(b2, np.float32),
        np.asarray(ow, np.float32), np.asarray(ob, np.float32),
    )
    xcT = np.concatenate(
        [towers.reshape(N * K, DT), x.reshape(N * K, D2)], axis=1
    ).T.astype(ml_dtypes.bfloat16)          # [128, N*K]
    in_maps = []
    for i in range(N_CORES):
        sl = slice(i * R, (i + 1) * R)
        m = {"cpack": cpack, "xc0T": np.ascontiguousarray(xcT[:, sl])}
        if FAST_OUT:
            m["idxs"] = _scatter_idxs()
        in_maps.append(m)
    return in_maps


def kernel(towers, x, w1, b1, w2, b2, aw1, ab1, aw2, ab2, ow, ob, k):
    global LAST_RESULT
    kk = int(k)

    if kk not in _PROGRAM_CACHE:
        nc = _build_program(kk)
        # encode pseudo ISA instructions (library reload) to raw ISA bytes;
        # walrus codegen rejects un-encoded pseudo instructions ("ISA wrong
        # length").  No effect on CoreSim timing or numerics.
        mybir.codegen_inst_isa_subclasses(nc)
        _PROGRAM_CACHE[kk] = nc
    nc = _PROGRAM_CACHE[kk]

    in_maps = _make_in_maps(towers, x, w1, b1, w2, b2, ow, ob)
    res = run_bass_kernel_spmd(nc, in_maps, list(range(N_CORES)))
    LAST_RESULT = res
    # out[g, 0] = product of graph g of that core
    out = np.concatenate([
        np.asarray(res.results[i]["out"])[:, 0].reshape(G)
        for i in range(N_CORES)
    ])
    return out.astype(np.float32)
